# revision 1
# baseline (speedup 1.0000x reference)
import sys
import numpy as np
import ml_dtypes

sys.path.insert(0, "/opt/trn_rl_repo")

import concourse.bass as bass
import concourse.tile as tile
from concourse import mybir
from concourse.bass_utils import run_bass_kernel_spmd

F32 = mybir.dt.float32
F32R = mybir.dt.float32r
BF16 = mybir.dt.bfloat16
AF = mybir.ActivationFunctionType
ALU = mybir.AluOpType

HID = 128
NT = 128       # tokens per image
NAH = 512      # atoms per core (half of 1024)
NG = 64        # ligand graphs
NI = 4         # images
NCORES = 8

TRACE = False
TRACE_KW = {}
LAST = None


_COMPUTE_INSTS = (
    "InstActivation", "InstTensorCopy", "InstTensorScalar", "InstTensorScalarPtr",
    "InstTensorTensor", "InstTensorTensorReduce", "InstTensorReduce", "InstMemSet",
    "InstMatmult", "InstScalarTensorTensor", "InstTensorTensorScan", "InstLdweights",
    "InstDMACopy", "InstDMATransposeAnt", "InstTriggeredCopy", "InstDrain",
    "InstEventSemaphoreOp", "InstSemaphoreOp", "InstCopy", "InstIota", "InstSelect",
)


def _legalize_waits(nc):
    # walrus in this toolchain accepts at most ONE sync wait on TPB compute
    # instructions; hoist extras into same-engine NoOps placed just before.
    k = 0
    for f in nc.m.functions:
        for blk in f.blocks:
            insts = blk.instructions
            out = []
            for ins in insts:
                si = getattr(ins, "sync_info", None)
                if (si is not None and len(si.on_wait) > 1
                        and type(ins).__name__ in _COMPUTE_INSTS):
                    waits = list(si.on_wait)
                    for w in waits[:-1]:
                        nop = mybir.InstNoOp(
                            name=f"WNOP-{k}", engine=ins.engine,
                            sync_info=mybir.SyncInfo(on_wait=[w], on_update=[]))
                        k += 1
                        out.append(nop)
                    ins.sync_info = mybir.SyncInfo(on_wait=[waits[-1]],
                                                   on_update=list(si.on_update))
                out.append(ins)
            blk.instructions = out
    return k


def build_program(bpe: float, bpg: float, bb2: float, bint_zero: bool = True, sim_trace: bool = False) -> bass.Bass:
    nc = bass.Bass()

    # ---- DRAM inputs (per-core views; same names across SPMD cores) ----
    d_tfT = nc.dram_tensor("tfT", [2, 128, 128], F32, kind="ExternalInput")
    d_laT = nc.dram_tensor("laT", [64, NAH], F32, kind="ExternalInput")
    d_lgT = nc.dram_tensor("lgT", [64, NG], F32, kind="ExternalInput")
    d_msf0 = nc.dram_tensor("msf0", [96, 4096], F32, kind="ExternalInput")
    d_msf1 = nc.dram_tensor("msf1", [64, 512], F32, kind="ExternalInput")
    d_S = nc.dram_tensor("Sh", [4, 128, NG], F32, kind="ExternalInput")

    d_wtok = nc.dram_tensor("W_token", [2, 128, HID], F32, kind="ExternalInput")
    d_w96 = nc.dram_tensor("W96", [9, 96, HID], F32, kind="ExternalInput")
    d_w0 = nc.dram_tensor("W0t", [27, 64, HID], F32, kind="ExternalInput")
    d_wpk = nc.dram_tensor("W_pocket", [2, 128, HID], F32, kind="ExternalInput")
    d_wcat = nc.dram_tensor("W_cat", [3, 128, HID], F32, kind="ExternalInput")
    d_wgate = nc.dram_tensor("W_gate", [3, 128, HID], F32, kind="ExternalInput")
    d_watom = nc.dram_tensor("W_atom", [64, HID], F32, kind="ExternalInput")
    d_wgraph = nc.dram_tensor("W_graph", [64, HID], F32, kind="ExternalInput")
    d_wb1 = nc.dram_tensor("W_bias1", [2, 128, HID], F32, kind="ExternalInput")
    d_wb2 = nc.dram_tensor("W_bias2", [128, 1], F32, kind="ExternalInput")
    d_wint = nc.dram_tensor("W_int", [128, HID], BF16, kind="ExternalInput")
    d_wpeg = nc.dram_tensor("W_peg", [128, 2], F32, kind="ExternalInput")

    d_btok = nc.dram_tensor("b_token", [128, 1], F32, kind="ExternalInput")
    d_bpk = nc.dram_tensor("b_pocket", [128, 1], F32, kind="ExternalInput")
    d_bcat = nc.dram_tensor("b_cat", [128, 1], F32, kind="ExternalInput")
    d_bgate = nc.dram_tensor("b_gate", [128, 1], F32, kind="ExternalInput")
    d_bgateh = nc.dram_tensor("b_gate_h", [128, 1], F32, kind="ExternalInput")
    d_batom = nc.dram_tensor("b_atom", [128, 1], F32, kind="ExternalInput")
    d_bgraph = nc.dram_tensor("b_graph", [128, 1], F32, kind="ExternalInput")
    d_bb1 = nc.dram_tensor("b_bias1", [128, 1], F32, kind="ExternalInput")
    d_bint = nc.dram_tensor("b_int", [128, 1], F32, kind="ExternalInput")

    d_seg = nc.dram_tensor("seg_out", [1, NG], F32, kind="ExternalOutput")
    d_bias = nc.dram_tensor("bias_out", [1, NG], F32, kind="ExternalOutput")

    tc_ref = tile.TileContext(nc, trace_sim=sim_trace)
    with tc_ref as tc:
        with (
            tc.tile_pool(name="const", bufs=1) as cpool,
            tc.tile_pool(name="pre_sb", bufs=2) as prepool,
            tc.tile_pool(name="big", bufs=1) as bigpool,
            tc.tile_pool(name="x", bufs=6) as xpool,
            tc.tile_pool(name="h", bufs=4) as hpool,
            tc.tile_pool(name="gate", bufs=4) as gpool,
            tc.tile_pool(name="junk", bufs=2) as jpool,
            tc.tile_pool(name="ps_pre", bufs=2, space="PSUM") as pspre,
            tc.tile_pool(name="ps_y", bufs=2, space="PSUM") as psy,
            tc.tile_pool(name="ps_z", bufs=2, space="PSUM") as psz,
        ):
            # ---------- loads ----------
            def load(pool, dram_ap, shape, tag, dt=F32):
                t = pool.tile(shape, dt, tag=tag)
                nc.sync.dma_start(t[:], dram_ap)
                return t

            def load_bias(pool, dram_ap, tag):
                tf_ = pool.tile([128, 1], F32, tag=tag + "_f")
                nc.sync.dma_start(tf_[:], dram_ap)
                t = pool.tile([128, 1], F32, tag=tag)
                nc.scalar.activation(t[:], tf_[:], AF.Copy)
                return t

            def cast_r(pool, src, shape, tag):
                t = pool.tile(shape, F32R, tag=tag)
                nc.scalar.activation(t[:], src[:], AF.Copy)
                return t

            wint = cpool.tile([128, HID], BF16, tag="wint")
            nc.sync.dma_start(wint[:], d_wint[:])
            wpeg_f = load(cpool, d_wpeg[:], [128, 2], "wpegf")
            wpeg = cast_r(cpool, wpeg_f, [128, 2], "wpeg")
            bint = load_bias(cpool, d_bint[:], "bint")
            watom_f = load(cpool, d_watom[:], [64, HID], "watomf")
            watom = cast_r(cpool, watom_f, [64, HID], "watom")
            batom = load_bias(cpool, d_batom[:], "batom")
            btok = load_bias(cpool, d_btok[:], "btok")
            wtok_f = cpool.tile([128, 2 * HID], F32, tag="wtokf")
            nc.sync.dma_start(wtok_f[:, 0:HID], d_wtok[0])
            nc.sync.dma_start(wtok_f[:, HID:2 * HID], d_wtok[1])
            wtok = cpool.tile([128, 2 * HID], F32R, tag="wtok")
            nc.scalar.activation(wtok[:, 0:HID], wtok_f[:, 0:HID], AF.Copy)
            nc.scalar.activation(wtok[:, HID:2 * HID], wtok_f[:, HID:2 * HID], AF.Copy)
            St = cpool.tile([128, 4 * NG], F32, tag="St")
            for q in range(4):
                nc.sync.dma_start(St[:, q * NG:(q + 1) * NG], d_S[q])

            # ---------- preamble: tok / atoms ----------
            tf = prepool.tile([128, 256], F32, tag="tf")
            nc.sync.dma_start(tf[:, 0:128], d_tfT[0])
            nc.sync.dma_start(tf[:, 128:256], d_tfT[1])
            # 2*silu(x) = x*(1+tanh(x/2)); factor 0.5 folded into W_token on host
            tft = prepool.tile([128, 256], F32, tag="tft")
            nc.scalar.activation(tft[:, 0:128], tf[:, 0:128], AF.Tanh, scale=0.5)
            nc.scalar.activation(tft[:, 128:256], tf[:, 128:256], AF.Tanh, scale=0.5)
            tfr = prepool.tile([128, 256], F32R, tag="tfr")
            nc.vector.scalar_tensor_tensor(tfr[:], tft[:], 1.0, tf[:],
                                           op0=ALU.add, op1=ALU.mult)
            ps_tok = pspre.tile([128, NT], F32, tag="ps")
            nc.tensor.matmul(ps_tok[:], wtok[:, 0:HID], tfr[:, 0:128], start=True, stop=False)
            nc.tensor.matmul(ps_tok[:], wtok[:, HID:2 * HID], tfr[:, 128:256], start=False, stop=True)
            tokT = cpool.tile([128, NT], F32, tag="tokT")
            nc.scalar.activation(tokT[:], ps_tok[:], AF.Identity, bias=btok[:])

            la_f = prepool.tile([64, NAH], F32, tag="laf")
            nc.sync.dma_start(la_f[:], d_laT[:])
            la = cast_r(prepool, la_f, [64, NAH], "la")
            ps_at = psy.tile([128, NAH], F32, tag="y")
            nc.tensor.matmul(ps_at[:], watom[:], la[:], start=True, stop=True)
            atomsT = cpool.tile([128, NAH], BF16, tag="atomsT")
            nc.scalar.activation(atomsT[:], ps_at[:], AF.Identity, bias=batom[:])

            # ---------- preamble: convs / pocket / pf / bias head ----------
            wpk = cpool.tile([128, 2 * HID], F32, tag="wpk")
            nc.sync.dma_start(wpk[:, 0:HID], d_wpk[0])
            nc.sync.dma_start(wpk[:, HID:2 * HID], d_wpk[1])
            wcat = cpool.tile([128, 3 * HID], F32, tag="wcat")
            wgate = cpool.tile([128, 3 * HID], F32, tag="wgate")
            for q in range(3):
                nc.sync.dma_start(wcat[:, q * HID:(q + 1) * HID], d_wcat[q])
                nc.sync.dma_start(wgate[:, q * HID:(q + 1) * HID], d_wgate[q])
            wgraph = load(cpool, d_wgraph[:], [64, HID], "wgraph")
            wb1 = cpool.tile([128, 2 * HID], F32, tag="wb1")
            nc.sync.dma_start(wb1[:, 0:HID], d_wb1[0])
            nc.sync.dma_start(wb1[:, HID:2 * HID], d_wb1[1])
            wb2 = load(cpool, d_wb2[:], [128, 1], "wb2")
            bpk = load_bias(cpool, d_bpk[:], "bpk")
            bcat = load_bias(cpool, d_bcat[:], "bcat")
            bgate = load_bias(cpool, d_bgate[:], "bgate")
            bgateh = load_bias(cpool, d_bgateh[:], "bgateh")
            bgraph = load_bias(cpool, d_bgraph[:], "bgraph")
            bb1 = load_bias(cpool, d_bb1[:], "bb1")
            w96_f = cpool.tile([96, 9 * HID], F32, tag="w96f")
            nc.sync.dma_start(w96_f[:, :].rearrange("p (t o) -> p t o", t=9),
                              d_w96[:, :, :].rearrange("t c o -> c t o"))
            w96 = cast_r(cpool, w96_f, [96, 9 * HID], "w96")
            w0_f = cpool.tile([64, 27 * HID], F32, tag="w0f")
            nc.sync.dma_start(w0_f[:, :].rearrange("p (t o) -> p t o", t=27),
                              d_w0[:, :, :].rearrange("t c o -> c t o"))
            w0 = cast_r(cpool, w0_f, [64, 27 * HID], "w0")

            # conv1 (ms_feat_0) — host sends 3x dx-shifted copies stacked on partitions
            x1f = bigpool.tile([96, 4096], F32, tag="x1f")
            nc.sync.dma_start(x1f[:], d_msf0[:])
            x1t = bigpool.tile([96, 4096], F32, tag="x1t")
            nc.scalar.activation(x1t[:], x1f[:], AF.Tanh, scale=0.5)
            x3 = bigpool.tile([96, 4096], F32R, tag="x3")
            nc.vector.scalar_tensor_tensor(x3[:], x1t[:], 1.0, x1f[:],
                                           op0=ALU.add, op1=ALU.mult)
            x3v = x3[:, :].rearrange("p (z q) -> p z q", z=16)
            x3v = x3v.rearrange("p z (b d) -> p z b d", b=16)  # [96, 16, 16, 16]

            x0f = prepool.tile([64, 512], F32, tag="x0f")
            nc.sync.dma_start(x0f[:], d_msf1[:])
            x0t = prepool.tile([64, 512], F32, tag="x0t")
            nc.scalar.activation(x0t[:], x0f[:], AF.Tanh, scale=0.5)
            x0 = prepool.tile([64, 512], F32R, tag="x0")
            nc.vector.scalar_tensor_tensor(x0[:], x0t[:], 1.0, x0f[:],
                                           op0=ALU.add, op1=ALU.mult)

            pre_tasks = []
            p1parts = prepool.tile([128, 7], F32, tag="p1p")
            def mk_conv1(c):
                def run():
                    ps_c1 = pspre.tile([128, 392], F32, tag="ps")
                    out_ap = ps_c1[:, :].rearrange("p (a b c) -> p a b c", a=2, b=14)
                    for dz in range(3):
                        for dy in range(3):
                            rhs = x3v[:, dz + 2 * c:dz + 2 * c + 2, dy:dy + 14, 0:14]
                            ti = dz * 3 + dy
                            nc.tensor.matmul(out_ap, w96[:, ti * HID:(ti + 1) * HID], rhs,
                                             start=(ti == 0), stop=(ti == 8))
                    junk = jpool.tile([128, 392], F32, tag="junk")
                    nc.vector.tensor_scalar(junk[:], ps_c1[:], 1.0, 0.0, op0=ALU.mult, op1=ALU.add,
                                            accum_out=p1parts[:, c:c + 1])
                return run
            for c in range(7):
                pre_tasks.append(mk_conv1(c))

            def task_conv0():
                ps_c0 = pspre.tile([128, 216], F32, tag="ps")
                out0_ap = ps_c0[:, :].rearrange("p (a b c) -> p a b c", a=6, b=6)
                x0v = x0[:, :].rearrange("p (z q) -> p z q", z=8)
                x0v = x0v.rearrange("p z (b d) -> p z b d", b=8)
                for dz in range(3):
                    for dy in range(3):
                        for dx in range(3):
                            rhs = x0v[:, dz:dz + 6, dy:dy + 6, dx:dx + 6]
                            ti = dz * 9 + dy * 3 + dx
                            nc.tensor.matmul(out0_ap, w0[:, ti * HID:(ti + 1) * HID], rhs,
                                             start=(ti == 0), stop=(ti == 26))
                p0 = prepool.tile([128, 1], F32, tag="p0")
                junk0 = jpool.tile([128, 216], F32, tag="junk")
                nc.vector.tensor_scalar(junk0[:], ps_c0[:], 1.0, 0.0, op0=ALU.mult, op1=ALU.add,
                                        accum_out=p0[:])
                p0m = prepool.tile([128, 1], F32, tag="p0m")
                nc.vector.tensor_scalar_mul(p0m[:], p0[:], 1.0 / 216.0)
                state["p0"] = p0; state["p0m"] = p0m
            pre_tasks.append(task_conv0)

            def task_pocket():
                p0, p0m = state["p0"], state["p0m"]
                p1 = prepool.tile([128, 1], F32, tag="p1")
                junk7 = jpool.tile([128, 7], F32, tag="junk7")
                nc.vector.tensor_scalar(junk7[:], p1parts[:], 1.0, 0.0, op0=ALU.mult, op1=ALU.add,
                                        accum_out=p1[:])
                p1m = prepool.tile([128, 1], F32, tag="p1m")
                nc.vector.tensor_scalar_mul(p1m[:], p1[:], 1.0 / 2744.0)
                tp0 = prepool.tile([128, 1], F32, tag="tp0")
                nc.scalar.activation(tp0[:], p0[:], AF.Tanh, scale=0.5 / 216.0)
                sp0 = prepool.tile([128, 1], F32, tag="sp0")
                nc.vector.scalar_tensor_tensor(sp0[:], tp0[:], 1.0, p0m[:],
                                               op0=ALU.add, op1=ALU.mult)
                tp1 = prepool.tile([128, 1], F32, tag="tp1")
                nc.scalar.activation(tp1[:], p1[:], AF.Tanh, scale=0.5 / 2744.0)
                sp1 = prepool.tile([128, 1], F32, tag="sp1")
                nc.vector.scalar_tensor_tensor(sp1[:], tp1[:], 1.0, p1m[:],
                                               op0=ALU.add, op1=ALU.mult)
                ps_pk = pspre.tile([128, 1], F32, tag="ps")
                nc.tensor.matmul(ps_pk[:], wpk[:, 0:HID], sp0[:], start=True, stop=False)
                nc.tensor.matmul(ps_pk[:], wpk[:, HID:2 * HID], sp1[:], start=False, stop=True)
                pocket = prepool.tile([128, 1], F32, tag="pocket")
                nc.scalar.activation(pocket[:], ps_pk[:], AF.Identity, bias=bpk[:])
                state["pocket"] = pocket
            pre_tasks.append(task_pocket)

            def task_pf():
                pocket = state["pocket"]
                tok_sum = prepool.tile([128, 1], F32, tag="toksum")
                junkt = jpool.tile([128, NT], F32, tag="junk")
                nc.vector.tensor_scalar(junkt[:], tokT[:], 1.0, 0.0, op0=ALU.mult, op1=ALU.add,
                                        accum_out=tok_sum[:])
                ps_pf = pspre.tile([128, 2], F32, tag="ps")
                chunks = [pocket, tok_sum, tok_sum]
                for q in range(3):
                    nc.tensor.matmul(ps_pf[:, 0:1], wcat[:, q * HID:(q + 1) * HID], chunks[q][:],
                                     start=(q == 0), stop=(q == 2))
                for q in range(3):
                    nc.tensor.matmul(ps_pf[:, 1:2], wgate[:, q * HID:(q + 1) * HID], chunks[q][:],
                                     start=(q == 0), stop=(q == 2))
                pf_t = prepool.tile([128, 1], F32, tag="pft")
                nc.scalar.activation(pf_t[:], ps_pf[:, 1:2], AF.Tanh, bias=bgateh[:], scale=0.5)
                pf_sig = prepool.tile([128, 1], F32, tag="pfsig")
                nc.vector.tensor_scalar(pf_sig[:], pf_t[:], 0.5, 0.5, op0=ALU.mult, op1=ALU.add)
                pf_lin = prepool.tile([128, 1], F32, tag="pflin")
                nc.scalar.activation(pf_lin[:], ps_pf[:, 0:1], AF.Identity, bias=bcat[:])
                pf = prepool.tile([128, 1], F32, tag="pf")
                nc.vector.tensor_mul(pf[:], pf_lin[:], pf_sig[:])
                state["pf"] = pf
            pre_tasks.append(task_pf)

            def task_bias():
                pf = state["pf"]
                lg = prepool.tile([64, NG], F32, tag="lg")
                nc.sync.dma_start(lg[:], d_lgT[:])
                ps_gf = pspre.tile([128, NG], F32, tag="ps")
                nc.tensor.matmul(ps_gf[:], wgraph[:], lg[:], start=True, stop=True)
                gfT = prepool.tile([128, NG], F32, tag="gfT")
                nc.scalar.activation(gfT[:], ps_gf[:], AF.Identity, bias=bgraph[:])
                ps_u = pspre.tile([128, 1], F32, tag="ps")
                nc.tensor.matmul(ps_u[:], wb1[:, 0:HID], pf[:], start=True, stop=True)
                ub = prepool.tile([128, 1], F32, tag="ub")
                nc.scalar.activation(ub[:], ps_u[:], AF.Identity, bias=bb1[:])
                ps_hb = pspre.tile([128, NG], F32, tag="ps")
                nc.tensor.matmul(ps_hb[:], wb1[:, HID:2 * HID], gfT[:], start=True, stop=True)
                hb = prepool.tile([128, NG], F32, tag="hb")
                nc.scalar.activation(hb[:], ps_hb[:], AF.Lrelu, bias=ub[:], alpha=0.01)
                ps_b2 = pspre.tile([1, NG], F32, tag="ps")
                nc.tensor.matmul(ps_b2[:], wb2[:], hb[:], start=True, stop=True)
                bias_sb = prepool.tile([1, NG], F32, tag="bias")
                nc.scalar.activation(bias_sb[:], ps_b2[:], AF.Identity, bias=bb2)
                nc.sync.dma_start(d_bias[:], bias_sb[:])
            pre_tasks.append(task_bias)
            state = {}

            # ---------- main loop ----------
            # acc[p, 8a + jt] accumulates pe for atom (128a + p), token group jt
            acc = cpool.tile([128, 32], F32, tag="acc")
            nc.vector.memset(acc[:], 0.0)
            zq4 = None
            for g in range(16):  # 8 tokens per group
                if g % 4 == 0:
                    zq4 = psz.tile([128, 256], F32, tag="z")  # 4 groups per bank
                zq = zq4[:, 64 * (g % 4):64 * (g % 4) + 64]  # col = 16a + 2jt + r
                for u in range(4):  # 2 tokens per u
                    y2 = psy.tile([128, 1024], F32, tag="y")
                    h2 = hpool.tile([128, 1024], F32R, tag="h")
                    for v in range(2):
                        j = 8 * g + 2 * u + v
                        x = xpool.tile([128, NAH], BF16, tag="x")
                        nc.vector.tensor_scalar_mul(x[:], atomsT[:], tokT[:, j:j + 1])
                        nc.tensor.matmul(y2[:, 512 * v:512 * (v + 1)], wint[:], x[:],
                                         start=True, stop=True)
                    if ((4 * g + u) % 6 != 5) or not bint_zero:
                        nc.scalar.activation(h2[:], y2[:], AF.Lrelu, bias=bint[:], alpha=0.01)
                    else:
                        # DVE leaky-relu (valid for b_int == 0): max(y, 0.01*y)
                        hscaled = hpool.tile([128, 1024], F32, tag="hs")
                        nc.vector.tensor_scalar_mul(hscaled[:], y2[:], 0.01)
                        nc.vector.tensor_max(h2[:], y2[:], hscaled[:])
                    for v in range(2):
                        jt = 2 * u + v
                        for a in range(4):
                            nc.tensor.matmul(zq[:, 16 * a + 2 * jt:16 * a + 2 * jt + 2],
                                             h2[:, 512 * v + 128 * a:512 * v + 128 * (a + 1)],
                                             wpeg[:], start=True, stop=True)
                # sigmoid(z1+bpg) = 0.5 + 0.5*tanh((z1+bpg)/2) -- tanh shares the
                # ACT table set with leaky_relu, so no table reloads in the loop
                s = gpool.tile([128, 32], F32, tag="s")
                nc.scalar.activation(s[:], zq[:, 1::2], AF.Tanh, bias=bpg * 0.5, scale=0.5)
                w = gpool.tile([128, 32], F32, tag="w")
                nc.vector.tensor_scalar(w[:], s[:], 0.5, 0.5, op0=ALU.mult, op1=ALU.add)
                t = gpool.tile([128, 32], F32, tag="t")
                nc.vector.scalar_tensor_tensor(t[:], zq[:, 0::2], bpe, w[:],
                                               op0=ALU.add, op1=ALU.mult)
                nc.vector.tensor_add(acc[:], acc[:], t[:])
                if g < len(pre_tasks):
                    pre_tasks[g]()

            # reduce over the 8 token-groups -> atom_e [128, 4] (atom chunks as cols)
            ae4 = prepool.tile([128, 4], F32, tag="ae4")
            junka = jpool.tile([128, 8], F32, tag="junk8")
            for a in range(4):
                junka = jpool.tile([128, 8], F32, tag="junk8")
                nc.vector.tensor_scalar(junka[:], acc[:, 8 * a:8 * (a + 1)], 1.0, 0.0,
                                        op0=ALU.mult, op1=ALU.add, accum_out=ae4[:, a:a + 1])
            ps_seg = pspre.tile([1, NG], F32, tag="ps")
            for q in range(4):
                nc.tensor.matmul(ps_seg[:], ae4[:, q:q + 1], St[:, q * NG:(q + 1) * NG],
                                 start=(q == 0), stop=(q == 3))
            seg_sb = prepool.tile([1, NG], F32, tag="seg")
            nc.scalar.activation(seg_sb[:], ps_seg[:], AF.Copy)
            nc.sync.dma_start(d_seg[:], seg_sb[:])


    _legalize_waits(nc)
    nc._tile_ctx = tc_ref
    return nc


def kernel(**inputs) -> np.ndarray:
    f = lambda a: np.ascontiguousarray(np.asarray(a), dtype=np.float32)
    tf = f(inputs["token_features"])
    la = f(inputs["lig_atom"])
    lg = f(inputs["lig_graph"])
    m0 = f(inputs["ms_feat_0"])
    m1 = f(inputs["ms_feat_1"])
    lb = np.asarray(inputs["ligand_batch"])
    S = (lb[:, None] == np.arange(NG)[None, :]).astype(np.float32)

    Wc1 = f(inputs["Wc1"])
    Wc0 = f(inputs["Wc0"])
    W96 = np.ascontiguousarray(Wc1.transpose(2, 3, 4, 1, 0).reshape(9, 96, HID))
    W0t = np.ascontiguousarray(Wc0.transpose(2, 3, 4, 1, 0).reshape(27, 64, HID))
    wcat = f(inputs["W_cat"]).copy()
    wgate = f(inputs["W_gate"]).copy()
    wcat[2 * HID:] /= float(NT)
    wgate[2 * HID:] /= float(NT)
    wpeg = np.concatenate([f(inputs["W_pe"]), f(inputs["W_pg"])], axis=1)

    bpe = float(np.asarray(inputs["b_pe"]).reshape(-1)[0])
    bpg = float(np.asarray(inputs["b_pg"]).reshape(-1)[0])
    bb2 = float(np.asarray(inputs["b_bias2"]).reshape(-1)[0])

    col = lambda a: f(a).reshape(128, 1)
    shared = {
        "W_token": (f(inputs["W_token"]) * 0.5).reshape(2, 128, HID),
        "W96": W96 * 0.5, "W0t": W0t * 0.5,
        "W_pocket": (f(inputs["W_pocket"]) * 0.5).reshape(2, 128, HID),
        "W_cat": wcat.reshape(3, 128, HID),
        "W_gate": wgate.reshape(3, 128, HID),
        "W_atom": f(inputs["W_atom"]),
        "W_graph": f(inputs["W_graph"]),
        "W_bias1": f(inputs["W_bias1"]).reshape(2, 128, HID),
        "W_bias2": f(inputs["W_bias2"]),
        "W_int": f(inputs["W_int"]).astype(ml_dtypes.bfloat16),
        "W_peg": wpeg,
        "b_token": col(inputs["b_token"]), "b_pocket": col(inputs["b_pocket"]),
        "b_cat": col(inputs["b_cat"]), "b_gate": col(inputs["b_gate"]),
        "b_atom": col(inputs["b_atom"]), "b_graph": col(inputs["b_graph"]),
        "b_bias1": col(inputs["b_bias1"]), "b_int": col(inputs["b_int"]),
        "b_gate_h": col(inputs["b_gate"]) * 0.5,
    }

    in_maps = []
    for c in range(NCORES):
        n, h = c // 2, c % 2
        m = dict(shared)
        m["tfT"] = np.ascontiguousarray(tf[n].T.reshape(2, 128, 128))
        m["laT"] = np.ascontiguousarray(la[n, 512 * h:512 * (h + 1)].T)
        m["lgT"] = np.ascontiguousarray(lg[n].T)
        m0f = m0[n].reshape(32, 4096)
        x3h = np.zeros((96, 4096), dtype=np.float32)
        for dd in range(3):
            x3h[32 * dd:32 * (dd + 1), 0:4096 - dd] = m0f[:, dd:]
        m["msf0"] = x3h
        m["msf1"] = m1[n].reshape(64, 512)
        m["Sh"] = np.ascontiguousarray(S[512 * h:512 * (h + 1)].reshape(4, 128, NG))
        in_maps.append(m)

    bint_zero = bool(np.all(np.asarray(inputs['b_int']) == 0.0))
    nc = build_program(bpe, bpg, bb2, bint_zero)
    r = run_bass_kernel_spmd(nc, in_maps, core_ids=list(range(NCORES)),
                             trace=TRACE, **(TRACE_KW if TRACE else {}))
    global LAST
    LAST = r
    res = r.results

    out = np.zeros((NI, NG), dtype=np.float32)
    for n in range(NI):
        out[n] = (res[2 * n]["seg_out"][0] + res[2 * n + 1]["seg_out"][0]
                  + res[2 * n]["bias_out"][0])
    return out



# revision 10
# speedup vs baseline: 1.7542x; 1.7542x over previous
import sys
import numpy as np
import ml_dtypes

sys.path.insert(0, "/opt/trn_rl_repo")

import concourse.bass as bass
import concourse.tile as tile
from concourse import mybir
from concourse.bass_utils import run_bass_kernel_spmd

F32 = mybir.dt.float32
F32R = mybir.dt.float32r
BF16 = mybir.dt.bfloat16
AF = mybir.ActivationFunctionType
ALU = mybir.AluOpType
AX = mybir.AxisListType

HID = 128
NT = 64        # tokens per core (half of 128)
NA = 1024      # atoms per core (all atoms)
NG = 64        # ligand graphs
NI = 4         # images
NCORES = 8

# engine for each in-loop leaky-relu chunk, indexed by token % 16:
# 'a' = ACT (scalar engine), 'v' = DVE (vector). (Pool/gpsimd cannot read
# PSUM, so the post-matmul leaky-relu can only run on ACT or DVE; Pool
# instead generates the per-token scaled weights Wj.)
LR_PATTERN = ['a', 'v', 'a', 'v', 'a', 'v', 'a', 'v',
              'a', 'v', 'a', 'v', 'a', 'v', 'a', 'a']

# f32 small-blob column indices
C_WPE, C_WPG, C_WB2 = 0, 1, 2
C_BTOK, C_BPK, C_BCAT, C_BGATEH = 3, 4, 5, 6
C_BATOM, C_BGRAPH, C_BB1, C_BINT, C_BPGH, C_BB2 = 7, 8, 9, 10, 11, 12
C_RPE, C_RPG = 13, 14
FW_COLS = 15

# bf16 weight-blob column offsets
O_WINT = 0            # [128, 128]
O_WTOK = 128          # 2 chunks of 128
O_WPK = 384           # 2 chunks
O_WCAT = 640          # 3 chunks
O_WGATE = 1024        # 3 chunks
O_WB1 = 1408          # 2 chunks
O_WAG = 1664          # watom rows 0:64
O_WG = 1792           # wgraph rows 0:64
BW_COLS = 1920

TRACE = False
TRACE_KW = {}
LAST = None


_COMPUTE_INSTS = (
    "InstActivation", "InstTensorCopy", "InstTensorScalar", "InstTensorScalarPtr",
    "InstTensorTensor", "InstTensorTensorReduce", "InstTensorReduce", "InstMemSet",
    "InstMatmult", "InstScalarTensorTensor", "InstTensorTensorScan", "InstLdweights",
    "InstDMACopy", "InstDMATransposeAnt", "InstTriggeredCopy", "InstDrain",
    "InstEventSemaphoreOp", "InstSemaphoreOp", "InstCopy", "InstIota", "InstSelect",
)


def _legalize_waits(nc):
    # walrus in this toolchain accepts at most ONE sync wait on TPB compute
    # instructions; hoist extras into same-engine NoOps placed just before.
    k = 0
    for f in nc.m.functions:
        for blk in f.blocks:
            insts = blk.instructions
            out = []
            for ins in insts:
                si = getattr(ins, "sync_info", None)
                if (si is not None and len(si.on_wait) > 1
                        and type(ins).__name__ in _COMPUTE_INSTS):
                    waits = list(si.on_wait)
                    for w in waits[:-1]:
                        nop = mybir.InstNoOp(
                            name=f"WNOP-{k}", engine=ins.engine,
                            sync_info=mybir.SyncInfo(on_wait=[w], on_update=[]))
                        k += 1
                        out.append(nop)
                    ins.sync_info = mybir.SyncInfo(on_wait=[waits[-1]],
                                                   on_update=list(si.on_update))
                out.append(ins)
            blk.instructions = out
    return k


def build_program(bpe: float, bpg: float, bb2: float, bint_zero: bool = True, sim_trace: bool = False) -> bass.Bass:
    nc = bass.Bass()

    # ---- DRAM inputs (per-core views; same names across SPMD cores) ----
    d_tf = nc.dram_tensor("tfT", [128, 256], BF16, kind="ExternalInput")
    d_la = nc.dram_tensor("laT", [64, NA], BF16, kind="ExternalInput")
    d_lg = nc.dram_tensor("lgT", [64, NG], BF16, kind="ExternalInput")
    d_m0 = nc.dram_tensor("msf0", [96, 4096], BF16, kind="ExternalInput")
    d_m1 = nc.dram_tensor("msf1", [64, 512], BF16, kind="ExternalInput")
    d_S = nc.dram_tensor("Sh", [128, 512], F32R, kind="ExternalInput")
    d_bw = nc.dram_tensor("BW", [128, BW_COLS], BF16, kind="ExternalInput")
    d_fw = nc.dram_tensor("FW", [128, FW_COLS], F32, kind="ExternalInput")
    d_wpeg = nc.dram_tensor("Wpeg", [128, 2], F32R, kind="ExternalInput")
    d_w96 = nc.dram_tensor("W96", [96, 9 * HID], BF16, kind="ExternalInput")
    d_w0 = nc.dram_tensor("W0t", [64, 27 * HID], BF16, kind="ExternalInput")

    d_out = nc.dram_tensor("out", [1, 128], F32, kind="ExternalOutput")

    tc_ref = tile.TileContext(nc, trace_sim=sim_trace)
    with tc_ref as tc:
        with (
            # f32r is bit-identical to f32; accumulating into f32r tiles is
            # full precision — this only silences the dtype-name check.
            nc.allow_low_precision(reason="f32r accumulators are fp32-width"),
            tc.tile_pool(name="const", bufs=1) as cpool,
            tc.tile_pool(name="pre_sb", bufs=2) as prepool,
            tc.tile_pool(name="wj", bufs=4) as wjpool,
            tc.tile_pool(name="h", bufs=4) as hpool,
            tc.tile_pool(name="gate", bufs=2) as gpool,
            tc.tile_pool(name="junk", bufs=2) as jpool,
            tc.tile_pool(name="ps_y", bufs=3, space="PSUM") as psy,
            tc.tile_pool(name="ps_z", bufs=1, space="PSUM") as psz,
            tc.tile_pool(name="ps_pre", bufs=1, space="PSUM") as pspre,
        ):
            # ---------- DMA loads (order = SP dispatch order) ----------
            tf = prepool.tile([128, 256], BF16, tag="tf")
            nc.sync.dma_start(tf[:], d_tf[:])
            fw = cpool.tile([128, FW_COLS], F32, tag="fw")
            nc.sync.dma_start(fw[:], d_fw[:])
            bw = cpool.tile([128, BW_COLS], BF16, tag="bw")
            nc.sync.dma_start(bw[:], d_bw[:])
            la = prepool.tile([64, NA], BF16, tag="la")
            nc.sync.dma_start(la[:], d_la[:])
            wpeg = cpool.tile([128, 2], F32R, tag="wpeg")
            nc.sync.dma_start(wpeg[:], d_wpeg[:])
            S = cpool.tile([128, 512], F32R, tag="S")
            nc.sync.dma_start(S[:], d_S[:])
            m1 = prepool.tile([64, 512], BF16, tag="m1")
            nc.sync.dma_start(m1[:], d_m1[:])
            lg = prepool.tile([64, NG], BF16, tag="lg")
            nc.sync.dma_start(lg[:], d_lg[:])
            w96 = cpool.tile([96, 9 * HID], BF16, tag="w96")
            nc.sync.dma_start(w96[:], d_w96[:])
            w0 = cpool.tile([64, 27 * HID], BF16, tag="w0")
            nc.sync.dma_start(w0[:], d_w0[:])
            m0 = cpool.tile([96, 4096], BF16, tag="m0")
            nc.sync.dma_start(m0[:], d_m0[:])

            wint = bw[:, O_WINT:O_WINT + 128]
            watom = bw[0:64, O_WAG:O_WAG + 128]
            wgraph = bw[0:64, O_WG:O_WG + 128]
            bcol = lambda c: fw[:, c:c + 1]

            # ---------- preamble: tok / atoms ----------
            # tfr = silu(tf) directly on ACT (Silu in same table as Prelu/Tanh)
            tfr = prepool.tile([128, 256], BF16, tag="tfr")
            nc.scalar.activation(tfr[:], tf[:], AF.Silu)
            ps_tok = pspre.tile([128, 128], F32, tag="ps")
            for q in range(2):
                nc.tensor.matmul(ps_tok[:], bw[:, O_WTOK + 128 * q:O_WTOK + 128 * (q + 1)],
                                 tfr[:, 128 * q:128 * (q + 1)],
                                 start=(q == 0), stop=(q == 1))
            tokT = cpool.tile([128, 128], F32, tag="tokT")
            nc.scalar.activation(tokT[:], ps_tok[:], AF.Identity, bias=bcol(C_BTOK))

            ps_at = psy.tile([128, NA], F32, tag="y")
            for v in range(2):
                nc.tensor.matmul(ps_at[:, 512 * v:512 * (v + 1)], watom,
                                 la[:, 512 * v:512 * (v + 1)], start=True, stop=True)
            atomsT = cpool.tile([128, NA], BF16, tag="atomsT")
            nc.scalar.activation(atomsT[:], ps_at[:], AF.Identity, bias=bcol(C_BATOM))

            # ---------- preamble tasks (interleaved at group boundaries) ----------
            state = {}
            x3 = cpool.tile([96, 4096], BF16, tag="x3")
            T1 = cpool.tile([96, 256], F32, tag="T1")
            s9 = cpool.tile([96, 9], BF16, tag="s9")
            x3v = x3[:, :].rearrange("p (zy x) -> p zy x", x=16)
            Tv = T1[:, :].rearrange("p (z y) -> p z y", z=16)

            def t_x3a():
                nc.scalar.activation(x3[:, 0:2048], m0[:, 0:2048], AF.Silu)

            def t_x3b():
                nc.scalar.activation(x3[:, 2048:4096], m0[:, 2048:4096], AF.Silu)
                x0 = prepool.tile([64, 512], BF16, tag="x0")
                nc.scalar.activation(x0[:], m1[:], AF.Silu)
                state["x0"] = x0

            def t_conv0():
                # conv0 on PE, bf16 moving -> 1 cyc/row
                x0 = state["x0"]
                x0v = x0[:, :].rearrange("p (z y x) -> p z y x", z=8, y=8)
                ps_c0 = pspre.tile([128, 216], F32, tag="ps")
                out0 = ps_c0[:, :].rearrange("p (a b c) -> p a b c", a=6, b=6)
                for dz in range(3):
                    for dy in range(3):
                        for dx in range(3):
                            ti = dz * 9 + dy * 3 + dx
                            rhs = x0v[:, dz:dz + 6, dy:dy + 6, dx:dx + 6]
                            nc.tensor.matmul(out0, w0[:, ti * HID:(ti + 1) * HID], rhs,
                                             start=(ti == 0), stop=(ti == 26))
                p0 = prepool.tile([128, 1], F32, tag="p0")
                nc.vector.tensor_reduce(p0[:], ps_c0[:], axis=AX.X, op=ALU.add)
                state["p0"] = p0
                # T-reduce first half (z in 0..7) needs only x3[:, 0:2048]
                nc.vector.tensor_reduce(T1[:, 0:128], x3v[:, 0:128, 0:14],
                                        axis=AX.X, op=ALU.add)

            def t_Tb():
                nc.vector.tensor_reduce(T1[:, 128:256], x3v[:, 128:256, 0:14],
                                        axis=AX.X, op=ALU.add)
                sp0 = prepool.tile([128, 1], BF16, tag="sp0")
                nc.scalar.activation(sp0[:], state["p0"][:], AF.Silu, scale=1.0 / 216.0)
                state["sp0"] = sp0

            def t_win_a():
                # first 5 conv1 windows: 1 on ACT (accum), 4 on DVE
                junk = jpool.tile([96, 196], F32, tag="junkw")
                nc.scalar.activation(junk[:], Tv[:, 0:14, 0:14], AF.Copy,
                                     accum_out=s9[:, 0:1])
                for w in range(1, 5):
                    dz, dy = w // 3, w % 3
                    nc.vector.tensor_reduce(s9[:, w:w + 1],
                                            Tv[:, dz:dz + 14, dy:dy + 14],
                                            axis=AX.XY, op=ALU.add)

            def t_win_b():
                junk = jpool.tile([96, 196], F32, tag="junkw")
                nc.scalar.activation(junk[:], Tv[:, 1:15, 2:16], AF.Copy,
                                     accum_out=s9[:, 5:6])
                for w in range(6, 9):
                    dz, dy = w // 3, w % 3
                    nc.vector.tensor_reduce(s9[:, w:w + 1],
                                            Tv[:, dz:dz + 14, dy:dy + 14],
                                            axis=AX.XY, op=ALU.add)
                p1 = pspre.tile([128, 1], F32, tag="ps")
                for t in range(9):
                    nc.tensor.matmul(p1[:], w96[:, t * HID:(t + 1) * HID],
                                     s9[:, t:t + 1], start=(t == 0), stop=(t == 8))
                sp1 = prepool.tile([128, 1], BF16, tag="sp1")
                nc.scalar.activation(sp1[:], p1[:], AF.Silu, scale=1.0 / 2744.0)
                state["sp1"] = sp1

            def t_pocket_pf():
                ps_pk = pspre.tile([128, 1], F32, tag="ps")
                nc.tensor.matmul(ps_pk[:], bw[:, O_WPK:O_WPK + 128], state["sp0"][:],
                                 start=True, stop=False)
                nc.tensor.matmul(ps_pk[:], bw[:, O_WPK + 128:O_WPK + 256], state["sp1"][:],
                                 start=False, stop=True)
                pocket = prepool.tile([128, 1], BF16, tag="pocket")
                nc.scalar.activation(pocket[:], ps_pk[:], AF.Identity, bias=bcol(C_BPK))
                tok_sum = prepool.tile([128, 1], BF16, tag="toksum")
                nc.vector.tensor_reduce(tok_sum[:], tokT[:], axis=AX.X, op=ALU.add)
                ps_pf = pspre.tile([128, 2], F32, tag="ps")
                chunks = [pocket, tok_sum, tok_sum]
                for q in range(3):
                    nc.tensor.matmul(ps_pf[:, 0:1], bw[:, O_WCAT + 128 * q:O_WCAT + 128 * (q + 1)],
                                     chunks[q][:], start=(q == 0), stop=(q == 2))
                for q in range(3):
                    nc.tensor.matmul(ps_pf[:, 1:2], bw[:, O_WGATE + 128 * q:O_WGATE + 128 * (q + 1)],
                                     chunks[q][:], start=(q == 0), stop=(q == 2))
                pft = prepool.tile([128, 1], F32, tag="pft")
                nc.scalar.activation(pft[:], ps_pf[:, 1:2], AF.Tanh,
                                     bias=bcol(C_BGATEH), scale=0.5)
                pfsig = prepool.tile([128, 1], F32, tag="pfsig")
                nc.scalar.activation(pfsig[:], pft[:], AF.Copy, bias=0.5, scale=0.5)
                pflin = prepool.tile([128, 1], F32, tag="pflin")
                nc.scalar.activation(pflin[:], ps_pf[:, 0:1], AF.Identity, bias=bcol(C_BCAT))
                pf = prepool.tile([128, 1], BF16, tag="pf")
                nc.vector.tensor_mul(pf[:], pflin[:], pfsig[:])
                state["pf"] = pf

            def t_bias():
                pf = state["pf"]
                ps_gf = pspre.tile([128, NG], F32, tag="ps")
                nc.tensor.matmul(ps_gf[:], wgraph, lg[:], start=True, stop=True)
                gfT = prepool.tile([128, NG], BF16, tag="gfT")
                nc.scalar.activation(gfT[:], ps_gf[:], AF.Identity, bias=bcol(C_BGRAPH))
                ps_u = pspre.tile([128, 1], F32, tag="ps")
                nc.tensor.matmul(ps_u[:], bw[:, O_WB1:O_WB1 + 128], pf[:],
                                 start=True, stop=True)
                ub = prepool.tile([128, 1], F32, tag="ub")
                nc.scalar.activation(ub[:], ps_u[:], AF.Identity, bias=bcol(C_BB1))
                ps_hb = pspre.tile([128, NG], F32, tag="ps")
                nc.tensor.matmul(ps_hb[:], bw[:, O_WB1 + 128:O_WB1 + 256], gfT[:],
                                 start=True, stop=True)
                hb = prepool.tile([128, NG], F32, tag="hb")
                nc.scalar.activation(hb[:], ps_hb[:], AF.Prelu, bias=ub[:], alpha=0.01)
                ps_b2 = pspre.tile([1, NG], F32, tag="ps")
                nc.tensor.matmul(ps_b2[:], fw[:, C_WB2:C_WB2 + 1], hb[:],
                                 start=True, stop=True)
                nc.scalar.activation(out_sb[:, 64:128], ps_b2[:], AF.Identity,
                                     bias=fw[0:1, C_BB2:C_BB2 + 1])

            pre_tasks = [t_x3a, t_x3b, t_conv0, t_Tb, t_win_a, t_win_b,
                         t_pocket_pf, t_bias]

            # ---------- main loop: 64 tokens in 8 groups of 8 ----------
            # leaky-relu is decomposed exactly: lrelu(v) = 0.99*relu(v) + 0.01*v.
            # The z matmuls consume relu(v) against wpeg pre-scaled by 0.99 on
            # the host; the 0.01*v part is linear, so its pe/pg contribution
            # lin_r[i,j] = sum_k a[k,i] * (0.01*r_r[k]) * tok[k,j] (with
            # r_r = W_int @ W_pe|W_pg from host FW cols) is accumulated into
            # the same PSUM banks by tiny matmuls issued before the z matmuls.
            out_sb = prepool.tile([1, 128], F32, tag="outsb")
            aep = cpool.tile([128, 32], F32, tag="aep")

            tokbf = cpool.tile([128, NT], BF16, tag="tokbf")
            nc.scalar.activation(tokbf[:], tokT[:, 0:NT], AF.Copy)
            ar = cpool.tile([128, 2 * NA], BF16, tag="ar")
            nc.vector.tensor_scalar_mul(ar[:, 0:NA], atomsT[:], bcol(C_RPE))
            nc.vector.tensor_scalar_mul(ar[:, NA:2 * NA], atomsT[:], bcol(C_RPG))

            zb = None
            for g in range(8):
                if g % 2 == 0:
                    b = g // 2
                    zb = psz.tile([128, 256], F32, tag="z")
                    # lin seed: zb[p, 128*gq + 16*a + 2*t + r] = lin_r[128a+p, j]
                    # (j = 16b + 8gq + t); the z matmuls then accumulate on top.
                    for gq in range(2):
                        for a in range(8):
                            for r in range(2):
                                base = 128 * gq + 16 * a
                                out_ap = zb[:, base + r:base + 16:2]
                                nc.tensor.matmul(
                                    out_ap,
                                    ar[:, NA * r + 128 * a:NA * r + 128 * (a + 1)],
                                    tokbf[:, 16 * b + 8 * gq:16 * b + 8 * (gq + 1)],
                                    start=True, stop=False, skip_group_check=True)
                for t in range(8):
                    j = 8 * g + t
                    wj = wjpool.tile([128, 128], BF16, tag="wj")
                    nc.gpsimd.tensor_scalar_mul(wj[:], wint, tokT[:, j:j + 1])
                    y = psy.tile([128, NA], F32, tag="y")
                    for v in range(2):
                        nc.tensor.matmul(y[:, 512 * v:512 * (v + 1)], wj[:],
                                         atomsT[:, 512 * v:512 * (v + 1)],
                                         start=True, stop=True)
                    h = hpool.tile([128, NA], F32R, tag="h")
                    if LR_PATTERN[j % 16] == 'a':
                        nc.scalar.activation(h[:], y[:], AF.Relu,
                                             bias=bcol(C_BINT))
                    else:
                        nc.vector.tensor_scalar(h[:], y[:], bcol(C_BINT), 0.0,
                                                op0=ALU.add, op1=ALU.max)
                    for a in range(8):
                        col = 128 * (g % 2) + 16 * a + 2 * t
                        nc.tensor.matmul(zb[:, col:col + 2],
                                         h[:, 128 * a:128 * (a + 1)], wpeg[:],
                                         start=False, stop=True,
                                         skip_group_check=True)
                if g % 2 == 1:
                    b = g // 2
                    s = gpool.tile([128, 128], F32, tag="s")
                    nc.scalar.activation(s[:], zb[:, 1::2], AF.Tanh,
                                         bias=bcol(C_BPGH), scale=0.5)
                    w = gpool.tile([128, 128], F32, tag="w")
                    nc.scalar.activation(w[:], s[:], AF.Copy, bias=0.5, scale=0.5)
                    t_ = gpool.tile([128, 128], F32, tag="t")
                    nc.vector.scalar_tensor_tensor(t_[:], zb[:, 0::2], bpe, w[:],
                                                   op0=ALU.add, op1=ALU.mult)
                    tv = t_[:, :].rearrange("p (gq a t) -> p a gq t", gq=2, a=8)
                    nc.vector.tensor_reduce(aep[:, 8 * b:8 * b + 8], tv,
                                            axis=AX.XY, op=ALU.add)
                if g < len(pre_tasks):
                    pre_tasks[g]()

            # ---------- tail: atom energies -> segments ----------
            ae8 = prepool.tile([128, 8], F32R, tag="ae8")
            nc.vector.tensor_reduce(ae8[:], aep[:, :].rearrange("p (b a) -> p a b", b=4),
                                    axis=AX.X, op=ALU.add)
            ps_seg = pspre.tile([1, NG], F32, tag="ps")
            for a in range(8):
                nc.tensor.matmul(ps_seg[:], ae8[:, a:a + 1], S[:, 64 * a:64 * (a + 1)],
                                 start=(a == 0), stop=(a == 7))
            nc.scalar.activation(out_sb[:, 0:64], ps_seg[:], AF.Copy)
            nc.sync.dma_start(d_out[:], out_sb[:])

    _legalize_waits(nc)
    nc._tile_ctx = tc_ref
    return nc


def kernel(**inputs) -> np.ndarray:
    f = lambda a: np.ascontiguousarray(np.asarray(a), dtype=np.float32)
    bf = lambda a: np.ascontiguousarray(np.asarray(a, dtype=np.float32)).astype(ml_dtypes.bfloat16)
    tf = f(inputs["token_features"])
    la = f(inputs["lig_atom"])
    lg = f(inputs["lig_graph"])
    m0 = f(inputs["ms_feat_0"])
    m1 = f(inputs["ms_feat_1"])
    lb = np.asarray(inputs["ligand_batch"])

    # one-hot segment matrix, atom-chunk-major: S[p, 64q+s] = [batch[128q+p]==s]
    S = (lb[:, None] == np.arange(NG)[None, :]).astype(np.float32)  # [1024, 64]
    Sh = np.zeros((128, 512), np.float32)
    for q in range(8):
        Sh[:, 64 * q:64 * (q + 1)] = S[128 * q:128 * (q + 1)]

    Wc1 = f(inputs["Wc1"])  # [128, 32, 3,3,3]
    Wc0 = f(inputs["Wc0"])  # [128, 64, 3,3,3]
    # W96[32*dx+c, 128*(3*dz+dy)+o] = Wc1[o,c,dz,dy,dx]
    W96 = Wc1.transpose(2, 3, 4, 1, 0).reshape(9, 96, HID)
    W96 = np.ascontiguousarray(W96.transpose(1, 0, 2).reshape(96, 9 * HID))
    W0t = Wc0.transpose(2, 3, 4, 1, 0).reshape(27, 64, HID)
    W0t = np.ascontiguousarray(W0t.transpose(1, 0, 2).reshape(64, 27 * HID))

    wcat = f(inputs["W_cat"]).copy()
    wgate = f(inputs["W_gate"]).copy()
    wcat[2 * HID:] /= 128.0   # token mean = sum / 128
    wgate[2 * HID:] /= 128.0

    # bf16 weight blob [128, BW_COLS]
    BW = np.zeros((128, BW_COLS), np.float32)
    BW[:, O_WINT:O_WINT + 128] = f(inputs["W_int"])
    BW[:, O_WTOK:O_WTOK + 256] = f(inputs["W_token"]).reshape(2, 128, HID).transpose(1, 0, 2).reshape(128, 256)
    BW[:, O_WPK:O_WPK + 256] = f(inputs["W_pocket"]).reshape(2, 128, HID).transpose(1, 0, 2).reshape(128, 256)
    BW[:, O_WCAT:O_WCAT + 384] = wcat.reshape(3, 128, HID).transpose(1, 0, 2).reshape(128, 384)
    BW[:, O_WGATE:O_WGATE + 384] = wgate.reshape(3, 128, HID).transpose(1, 0, 2).reshape(128, 384)
    BW[:, O_WB1:O_WB1 + 256] = f(inputs["W_bias1"]).reshape(2, 128, HID).transpose(1, 0, 2).reshape(128, 256)
    BW[0:64, O_WAG:O_WAG + 128] = f(inputs["W_atom"])
    BW[0:64, O_WG:O_WG + 128] = f(inputs["W_graph"])

    # f32 small blob [128, FW_COLS]
    col = lambda a: f(a).reshape(-1)
    FW = np.zeros((128, FW_COLS), np.float32)
    FW[:, C_WPE] = col(inputs["W_pe"])
    FW[:, C_WPG] = col(inputs["W_pg"])
    FW[:, C_WB2] = col(inputs["W_bias2"])
    FW[:, C_BTOK] = col(inputs["b_token"])
    FW[:, C_BPK] = col(inputs["b_pocket"])
    FW[:, C_BCAT] = col(inputs["b_cat"])
    FW[:, C_BGATEH] = col(inputs["b_gate"]) * 0.5
    FW[:, C_BATOM] = col(inputs["b_atom"])
    FW[:, C_BGRAPH] = col(inputs["b_graph"])
    FW[:, C_BB1] = col(inputs["b_bias1"])
    FW[:, C_BINT] = col(inputs["b_int"])
    wpe_v = f(inputs["W_pe"]).reshape(-1)
    wpg_v = f(inputs["W_pg"]).reshape(-1)
    wint_f = f(inputs["W_int"])
    bint_v = col(inputs["b_int"])
    FW[:, C_RPE] = 0.01 * (wint_f @ wpe_v)
    FW[:, C_RPG] = 0.01 * (wint_f @ wpg_v)
    bpe_eff = float(np.asarray(inputs["b_pe"]).reshape(-1)[0]) + 0.01 * float(wpe_v @ bint_v)
    bpg_eff = float(np.asarray(inputs["b_pg"]).reshape(-1)[0]) + 0.01 * float(wpg_v @ bint_v)
    FW[:, C_BPGH] = bpg_eff * 0.5
    FW[:, C_BB2] = float(np.asarray(inputs["b_bias2"]).reshape(-1)[0])

    Wpeg = 0.99 * np.concatenate([f(inputs["W_pe"]).reshape(128, 1),
                                  f(inputs["W_pg"]).reshape(128, 1)], axis=1)

    bpe = bpe_eff
    bpg = bpg_eff
    bb2 = float(np.asarray(inputs["b_bias2"]).reshape(-1)[0])

    shared = {
        "BW": BW.astype(ml_dtypes.bfloat16),
        "FW": FW,
        "Wpeg": Wpeg,
        "W96": W96.astype(ml_dtypes.bfloat16),
        "W0t": W0t.astype(ml_dtypes.bfloat16),
        "Sh": Sh,
    }

    in_maps = []
    for c in range(NCORES):
        n, h = c // 2, c % 2
        m = dict(shared)
        # permute tokens: this core's 64 first
        perm = np.concatenate([np.arange(64 * h, 64 * (h + 1)),
                               np.arange(64 * (1 - h), 64 * (2 - h))])
        tfp = tf[n][perm]                       # [128 tok, 256 feat]
        m["tfT"] = bf(np.ascontiguousarray(tfp.T))   # [256, 128] -> [128,256] view below
        # note: dram is [128, 256] = 2 chunks of features stacked on cols
        m["tfT"] = bf(np.ascontiguousarray(tfp.T.reshape(2, 128, 128).transpose(1, 0, 2).reshape(128, 256)))
        m["laT"] = bf(la[n].T)                  # [64, 1024]
        m["lgT"] = bf(lg[n].T)                  # [64, 64]
        m0f = m0[n].reshape(32, 4096)
        x3h = np.zeros((96, 4096), dtype=np.float32)
        for dd in range(3):
            x3h[32 * dd:32 * (dd + 1), 0:4096 - dd] = m0f[:, dd:]
        m["msf0"] = bf(x3h)
        m["msf1"] = bf(m1[n].reshape(64, 512))
        in_maps.append(m)

    bint_zero = bool(np.all(np.asarray(inputs['b_int']) == 0.0))
    nc = build_program(bpe, bpg, bb2, bint_zero)
    r = run_bass_kernel_spmd(nc, in_maps, core_ids=list(range(NCORES)),
                             trace=TRACE, **(TRACE_KW if TRACE else {}))
    global LAST
    LAST = r
    res = r.results

    out = np.zeros((NI, NG), dtype=np.float32)
    for n in range(NI):
        out[n] = (res[2 * n]["out"][0, 0:64] + res[2 * n + 1]["out"][0, 0:64]
                  + res[2 * n]["out"][0, 64:128])
    return out


# revision 11
# speedup vs baseline: 1.8418x; 1.0499x over previous
import sys
import numpy as np
import ml_dtypes

sys.path.insert(0, "/opt/trn_rl_repo")

import concourse.bass as bass
import concourse.tile as tile
from concourse import mybir
from concourse.bass_utils import run_bass_kernel_spmd

F32 = mybir.dt.float32
F32R = mybir.dt.float32r
BF16 = mybir.dt.bfloat16
AF = mybir.ActivationFunctionType
ALU = mybir.AluOpType
AX = mybir.AxisListType

HID = 128
NT = 64        # tokens per core (half of 128)
NA = 1024      # atoms per core (all atoms)
NG = 64        # ligand graphs
NI = 4         # images
NCORES = 8

# engine for each in-loop leaky-relu chunk, indexed by token % 16:
# 'a' = ACT (scalar engine), 'v' = DVE (vector). (Pool/gpsimd cannot read
# PSUM, so the post-matmul leaky-relu can only run on ACT or DVE; Pool
# instead generates the per-token scaled weights Wj.)
LR_PATTERN = ['a', 'v', 'a', 'v', 'a', 'v', 'a', 'v',
              'a', 'v', 'a', 'v', 'a', 'v', 'a', 'v',
              'a', 'v', 'a', 'v', 'a', 'v', 'a', 'v',
              'a', 'v', 'a', 'v', 'a', 'a', 'a', 'a']

# f32 small-blob column indices
C_WPE, C_WPG, C_WB2 = 0, 1, 2
C_BTOK, C_BPK, C_BCAT, C_BGATEH = 3, 4, 5, 6
C_BATOM, C_BGRAPH, C_BB1, C_BINT, C_BPGH, C_BB2 = 7, 8, 9, 10, 11, 12
C_RPE, C_RPG = 13, 14
FW_COLS = 15

# bf16 weight-blob column offsets
O_WINT = 0            # [128, 128]
O_WTOK = 128          # 2 chunks of 128
O_WPK = 384           # 2 chunks
O_WCAT = 640          # 3 chunks
O_WGATE = 1024        # 3 chunks
O_WB1 = 1408          # 2 chunks
O_WAG = 1664          # watom rows 0:64
O_WG = 1792           # wgraph rows 0:64
BW_COLS = 1920

TRACE = False
TRACE_KW = {}
LAST = None


_COMPUTE_INSTS = (
    "InstActivation", "InstTensorCopy", "InstTensorScalar", "InstTensorScalarPtr",
    "InstTensorTensor", "InstTensorTensorReduce", "InstTensorReduce", "InstMemSet",
    "InstMatmult", "InstScalarTensorTensor", "InstTensorTensorScan", "InstLdweights",
    "InstDMACopy", "InstDMATransposeAnt", "InstTriggeredCopy", "InstDrain",
    "InstEventSemaphoreOp", "InstSemaphoreOp", "InstCopy", "InstIota", "InstSelect",
)


def _legalize_waits(nc):
    # walrus in this toolchain accepts at most ONE sync wait on TPB compute
    # instructions; hoist extras into same-engine NoOps placed just before.
    k = 0
    for f in nc.m.functions:
        for blk in f.blocks:
            insts = blk.instructions
            out = []
            for ins in insts:
                si = getattr(ins, "sync_info", None)
                if (si is not None and len(si.on_wait) > 1
                        and type(ins).__name__ in _COMPUTE_INSTS):
                    waits = list(si.on_wait)
                    for w in waits[:-1]:
                        nop = mybir.InstNoOp(
                            name=f"WNOP-{k}", engine=ins.engine,
                            sync_info=mybir.SyncInfo(on_wait=[w], on_update=[]))
                        k += 1
                        out.append(nop)
                    ins.sync_info = mybir.SyncInfo(on_wait=[waits[-1]],
                                                   on_update=list(si.on_update))
                out.append(ins)
            blk.instructions = out
    return k


def build_program(bpe: float, bpg: float, bb2: float, bint_zero: bool = True, sim_trace: bool = False) -> bass.Bass:
    nc = bass.Bass()

    # ---- DRAM inputs (per-core views; same names across SPMD cores) ----
    d_tf = nc.dram_tensor("tfT", [128, 256], BF16, kind="ExternalInput")
    d_la = nc.dram_tensor("laT", [64, NA], BF16, kind="ExternalInput")
    d_lg = nc.dram_tensor("lgT", [64, NG], BF16, kind="ExternalInput")
    d_m0 = nc.dram_tensor("msf0", [96, 4096], BF16, kind="ExternalInput")
    d_m1 = nc.dram_tensor("msf1", [64, 512], BF16, kind="ExternalInput")
    d_S = nc.dram_tensor("Sh", [128, 512], F32R, kind="ExternalInput")
    d_bw = nc.dram_tensor("BW", [128, BW_COLS], BF16, kind="ExternalInput")
    d_fw = nc.dram_tensor("FW", [128, FW_COLS], F32, kind="ExternalInput")
    d_wpeg = nc.dram_tensor("Wpeg", [128, 2], F32R, kind="ExternalInput")
    d_w96 = nc.dram_tensor("W96", [96, 9 * HID], BF16, kind="ExternalInput")
    d_w0 = nc.dram_tensor("W0t", [64, 27 * HID], BF16, kind="ExternalInput")

    d_out = nc.dram_tensor("out", [1, 128], F32, kind="ExternalOutput")

    tc_ref = tile.TileContext(nc, trace_sim=sim_trace)
    with tc_ref as tc:
        with (
            # f32r is bit-identical to f32; accumulating into f32r tiles is
            # full precision — this only silences the dtype-name check.
            nc.allow_low_precision(reason="f32r accumulators are fp32-width"),
            tc.tile_pool(name="const", bufs=1) as cpool,
            tc.tile_pool(name="pre_sb", bufs=2) as prepool,
            tc.tile_pool(name="wj", bufs=4) as wjpool,
            tc.tile_pool(name="h", bufs=4) as hpool,
            tc.tile_pool(name="gate", bufs=2) as gpool,
            tc.tile_pool(name="junk", bufs=2) as jpool,
            tc.tile_pool(name="ps_y", bufs=3, space="PSUM") as psy,
            tc.tile_pool(name="ps_z", bufs=1, space="PSUM") as psz,
            tc.tile_pool(name="ps_pre", bufs=1, space="PSUM") as pspre,
        ):
            # ---------- DMA loads (order = SP dispatch order) ----------
            tf = prepool.tile([128, 256], BF16, tag="tf")
            nc.sync.dma_start(tf[:], d_tf[:])
            fw = cpool.tile([128, FW_COLS], F32, tag="fw")
            nc.sync.dma_start(fw[:], d_fw[:])
            bw = cpool.tile([128, BW_COLS], BF16, tag="bw")
            nc.sync.dma_start(bw[:], d_bw[:])
            la = prepool.tile([64, NA], BF16, tag="la")
            nc.sync.dma_start(la[:], d_la[:])
            wpeg = cpool.tile([128, 2], F32R, tag="wpeg")
            nc.sync.dma_start(wpeg[:], d_wpeg[:])
            S = cpool.tile([128, 512], F32R, tag="S")
            nc.sync.dma_start(S[:], d_S[:])
            m1 = prepool.tile([64, 512], BF16, tag="m1")
            nc.sync.dma_start(m1[:], d_m1[:])
            lg = prepool.tile([64, NG], BF16, tag="lg")
            nc.sync.dma_start(lg[:], d_lg[:])
            w96 = cpool.tile([96, 9 * HID], BF16, tag="w96")
            nc.sync.dma_start(w96[:], d_w96[:])
            w0 = cpool.tile([64, 27 * HID], BF16, tag="w0")
            nc.sync.dma_start(w0[:], d_w0[:])
            m0 = cpool.tile([96, 4096], BF16, tag="m0")
            nc.sync.dma_start(m0[:], d_m0[:])

            wint = bw[:, O_WINT:O_WINT + 128]
            watom = bw[0:64, O_WAG:O_WAG + 128]
            wgraph = bw[0:64, O_WG:O_WG + 128]
            bcol = lambda c: fw[:, c:c + 1]

            # ---------- preamble: tok / atoms ----------
            # tfr = silu(tf) directly on ACT (Silu in same table as Prelu/Tanh)
            tfr = prepool.tile([128, 256], BF16, tag="tfr")
            nc.scalar.activation(tfr[:], tf[:], AF.Silu)
            ps_tok = pspre.tile([128, 128], F32, tag="ps")
            for q in range(2):
                nc.tensor.matmul(ps_tok[:], bw[:, O_WTOK + 128 * q:O_WTOK + 128 * (q + 1)],
                                 tfr[:, 128 * q:128 * (q + 1)],
                                 start=(q == 0), stop=(q == 1))
            tokT = cpool.tile([128, 128], F32, tag="tokT")
            nc.scalar.activation(tokT[:], ps_tok[:], AF.Identity, bias=bcol(C_BTOK))

            ps_at = psy.tile([128, NA], F32, tag="y")
            for v in range(2):
                nc.tensor.matmul(ps_at[:, 512 * v:512 * (v + 1)], watom,
                                 la[:, 512 * v:512 * (v + 1)], start=True, stop=True)
            atomsT = cpool.tile([128, NA], BF16, tag="atomsT")
            nc.scalar.activation(atomsT[:], ps_at[:], AF.Identity, bias=bcol(C_BATOM))

            # ---------- preamble tasks (interleaved at group boundaries) ----------
            state = {}
            x3 = cpool.tile([96, 4096], BF16, tag="x3")
            T1 = cpool.tile([96, 256], F32, tag="T1")
            s9 = cpool.tile([96, 9], BF16, tag="s9")
            x3v = x3[:, :].rearrange("p (zy x) -> p zy x", x=16)
            Tv = T1[:, :].rearrange("p (z y) -> p z y", z=16)

            def t_x3a():
                nc.scalar.activation(x3[:, 0:2048], m0[:, 0:2048], AF.Silu)

            def t_x3b():
                nc.scalar.activation(x3[:, 2048:4096], m0[:, 2048:4096], AF.Silu)
                x0 = prepool.tile([64, 512], BF16, tag="x0")
                nc.scalar.activation(x0[:], m1[:], AF.Silu)
                state["x0"] = x0

            def t_conv0():
                # conv0 on PE, bf16 moving -> 1 cyc/row
                x0 = state["x0"]
                x0v = x0[:, :].rearrange("p (z y x) -> p z y x", z=8, y=8)
                ps_c0 = pspre.tile([128, 216], F32, tag="ps")
                out0 = ps_c0[:, :].rearrange("p (a b c) -> p a b c", a=6, b=6)
                for dz in range(3):
                    for dy in range(3):
                        for dx in range(3):
                            ti = dz * 9 + dy * 3 + dx
                            rhs = x0v[:, dz:dz + 6, dy:dy + 6, dx:dx + 6]
                            nc.tensor.matmul(out0, w0[:, ti * HID:(ti + 1) * HID], rhs,
                                             start=(ti == 0), stop=(ti == 26))
                p0 = prepool.tile([128, 1], F32, tag="p0")
                nc.vector.tensor_reduce(p0[:], ps_c0[:], axis=AX.X, op=ALU.add)
                state["p0"] = p0
                # T-reduce first half (z in 0..7) needs only x3[:, 0:2048]
                nc.vector.tensor_reduce(T1[:, 0:128], x3v[:, 0:128, 0:14],
                                        axis=AX.X, op=ALU.add)

            def t_Tb():
                nc.vector.tensor_reduce(T1[:, 128:256], x3v[:, 128:256, 0:14],
                                        axis=AX.X, op=ALU.add)
                sp0 = prepool.tile([128, 1], BF16, tag="sp0")
                nc.scalar.activation(sp0[:], state["p0"][:], AF.Silu, scale=1.0 / 216.0)
                state["sp0"] = sp0

            def t_win_a():
                # first 5 conv1 windows: 1 on ACT (accum), 4 on DVE
                junk = jpool.tile([96, 196], F32, tag="junkw")
                nc.scalar.activation(junk[:], Tv[:, 0:14, 0:14], AF.Copy,
                                     accum_out=s9[:, 0:1])
                for w in range(1, 5):
                    dz, dy = w // 3, w % 3
                    nc.vector.tensor_reduce(s9[:, w:w + 1],
                                            Tv[:, dz:dz + 14, dy:dy + 14],
                                            axis=AX.XY, op=ALU.add)

            def t_win_b():
                junk = jpool.tile([96, 196], F32, tag="junkw")
                nc.scalar.activation(junk[:], Tv[:, 1:15, 2:16], AF.Copy,
                                     accum_out=s9[:, 5:6])
                for w in range(6, 9):
                    dz, dy = w // 3, w % 3
                    nc.vector.tensor_reduce(s9[:, w:w + 1],
                                            Tv[:, dz:dz + 14, dy:dy + 14],
                                            axis=AX.XY, op=ALU.add)
                p1 = pspre.tile([128, 1], F32, tag="ps")
                for t in range(9):
                    nc.tensor.matmul(p1[:], w96[:, t * HID:(t + 1) * HID],
                                     s9[:, t:t + 1], start=(t == 0), stop=(t == 8))
                sp1 = prepool.tile([128, 1], BF16, tag="sp1")
                nc.scalar.activation(sp1[:], p1[:], AF.Silu, scale=1.0 / 2744.0)
                state["sp1"] = sp1

            def t_pocket_pf():
                ps_pk = pspre.tile([128, 1], F32, tag="ps")
                nc.tensor.matmul(ps_pk[:], bw[:, O_WPK:O_WPK + 128], state["sp0"][:],
                                 start=True, stop=False)
                nc.tensor.matmul(ps_pk[:], bw[:, O_WPK + 128:O_WPK + 256], state["sp1"][:],
                                 start=False, stop=True)
                pocket = prepool.tile([128, 1], BF16, tag="pocket")
                nc.scalar.activation(pocket[:], ps_pk[:], AF.Identity, bias=bcol(C_BPK))
                tok_sum = prepool.tile([128, 1], BF16, tag="toksum")
                nc.vector.tensor_reduce(tok_sum[:], tokT[:], axis=AX.X, op=ALU.add)
                ps_pf = pspre.tile([128, 2], F32, tag="ps")
                chunks = [pocket, tok_sum, tok_sum]
                for q in range(3):
                    nc.tensor.matmul(ps_pf[:, 0:1], bw[:, O_WCAT + 128 * q:O_WCAT + 128 * (q + 1)],
                                     chunks[q][:], start=(q == 0), stop=(q == 2))
                for q in range(3):
                    nc.tensor.matmul(ps_pf[:, 1:2], bw[:, O_WGATE + 128 * q:O_WGATE + 128 * (q + 1)],
                                     chunks[q][:], start=(q == 0), stop=(q == 2))
                pft = prepool.tile([128, 1], F32, tag="pft")
                nc.scalar.activation(pft[:], ps_pf[:, 1:2], AF.Tanh,
                                     bias=bcol(C_BGATEH), scale=0.5)
                pfsig = prepool.tile([128, 1], F32, tag="pfsig")
                nc.scalar.activation(pfsig[:], pft[:], AF.Copy, bias=0.5, scale=0.5)
                pflin = prepool.tile([128, 1], F32, tag="pflin")
                nc.scalar.activation(pflin[:], ps_pf[:, 0:1], AF.Identity, bias=bcol(C_BCAT))
                pf = prepool.tile([128, 1], BF16, tag="pf")
                nc.vector.tensor_mul(pf[:], pflin[:], pfsig[:])
                state["pf"] = pf

            def t_bias():
                pf = state["pf"]
                ps_gf = pspre.tile([128, NG], F32, tag="ps")
                nc.tensor.matmul(ps_gf[:], wgraph, lg[:], start=True, stop=True)
                gfT = prepool.tile([128, NG], BF16, tag="gfT")
                nc.scalar.activation(gfT[:], ps_gf[:], AF.Identity, bias=bcol(C_BGRAPH))
                ps_u = pspre.tile([128, 1], F32, tag="ps")
                nc.tensor.matmul(ps_u[:], bw[:, O_WB1:O_WB1 + 128], pf[:],
                                 start=True, stop=True)
                ub = prepool.tile([128, 1], F32, tag="ub")
                nc.scalar.activation(ub[:], ps_u[:], AF.Identity, bias=bcol(C_BB1))
                ps_hb = pspre.tile([128, NG], F32, tag="ps")
                nc.tensor.matmul(ps_hb[:], bw[:, O_WB1 + 128:O_WB1 + 256], gfT[:],
                                 start=True, stop=True)
                hb = prepool.tile([128, NG], F32, tag="hb")
                nc.scalar.activation(hb[:], ps_hb[:], AF.Prelu, bias=ub[:], alpha=0.01)
                ps_b2 = pspre.tile([1, NG], F32, tag="ps")
                nc.tensor.matmul(ps_b2[:], fw[:, C_WB2:C_WB2 + 1], hb[:],
                                 start=True, stop=True)
                nc.scalar.activation(out_sb[:, 64:128], ps_b2[:], AF.Identity,
                                     bias=fw[0:1, C_BB2:C_BB2 + 1])

            pre_tasks = [t_x3a, t_x3b, t_conv0, t_Tb, t_win_a, t_win_b,
                         t_pocket_pf, t_bias]

            # ---------- main loop: 64 tokens in 8 groups of 8 ----------
            # leaky-relu is decomposed exactly: lrelu(v) = 0.99*relu(v) + 0.01*v.
            # The z matmuls consume relu(v) against wpeg pre-scaled by 0.99 on
            # the host; the 0.01*v part is linear, so its pe/pg contribution
            # lin_r[i,j] = sum_k a[k,i] * (0.01*r_r[k]) * tok[k,j] (with
            # r_r = W_int @ W_pe|W_pg from host FW cols) is accumulated into
            # the same PSUM banks by tiny matmuls issued before the z matmuls.
            out_sb = prepool.tile([1, 128], F32, tag="outsb")
            aep = cpool.tile([128, 32], F32, tag="aep")

            tokbf = cpool.tile([128, NT], BF16, tag="tokbf")
            nc.scalar.activation(tokbf[:], tokT[:, 0:NT], AF.Copy)
            ar = cpool.tile([128, 2 * NA], BF16, tag="ar")
            nc.vector.tensor_scalar_mul(ar[:, 0:NA], atomsT[:], bcol(C_RPE))
            nc.vector.tensor_scalar_mul(ar[:, NA:2 * NA], atomsT[:], bcol(C_RPG))

            zb = None
            for g in range(8):
                if g % 2 == 0:
                    b = g // 2
                    zb = psz.tile([128, 256], F32, tag="z")
                    # lin seed: zb[p, 128*gq + 16*a + 2*t + r] = lin_r[128a+p, j]
                    # (j = 16b + 8gq + t); the z matmuls then accumulate on top.
                    for gq in range(2):
                        for a in range(8):
                            for r in range(2):
                                base = 128 * gq + 16 * a
                                out_ap = zb[:, base + r:base + 16:2]
                                nc.tensor.matmul(
                                    out_ap,
                                    ar[:, NA * r + 128 * a:NA * r + 128 * (a + 1)],
                                    tokbf[:, 16 * b + 8 * gq:16 * b + 8 * (gq + 1)],
                                    start=True, stop=False, skip_group_check=True)
                for t in range(8):
                    j = 8 * g + t
                    wj = wjpool.tile([128, 128], BF16, tag="wj")
                    nc.gpsimd.tensor_scalar_mul(wj[:], wint, tokT[:, j:j + 1])
                    y = psy.tile([128, NA], F32, tag="y")
                    for v in range(2):
                        nc.tensor.matmul(y[:, 512 * v:512 * (v + 1)], wj[:],
                                         atomsT[:, 512 * v:512 * (v + 1)],
                                         start=True, stop=True)
                    h = hpool.tile([128, NA], F32R, tag="h")
                    if LR_PATTERN[j % 32] == 'a':
                        nc.scalar.activation(h[:], y[:], AF.Relu,
                                             bias=bcol(C_BINT))
                    else:
                        nc.vector.tensor_scalar(h[:], y[:], bcol(C_BINT), 0.0,
                                                op0=ALU.add, op1=ALU.max)
                    for a in range(8):
                        col = 128 * (g % 2) + 16 * a + 2 * t
                        nc.tensor.matmul(zb[:, col:col + 2],
                                         h[:, 128 * a:128 * (a + 1)], wpeg[:],
                                         start=False, stop=True,
                                         skip_group_check=True)
                if g % 2 == 1:
                    b = g // 2
                    s = gpool.tile([128, 128], F32, tag="s")
                    nc.scalar.activation(s[:], zb[:, 1::2], AF.Tanh,
                                         bias=bcol(C_BPGH), scale=0.5)
                    w = gpool.tile([128, 128], F32, tag="w")
                    nc.gpsimd.tensor_scalar(w[:], s[:], 0.5, 0.5,
                                            op0=ALU.mult, op1=ALU.add)
                    t_ = gpool.tile([128, 128], F32, tag="t")
                    nc.vector.scalar_tensor_tensor(t_[:], zb[:, 0::2], bpe, w[:],
                                                   op0=ALU.add, op1=ALU.mult)
                    tv = t_[:, :].rearrange("p (gq a t) -> p a gq t", gq=2, a=8)
                    nc.vector.tensor_reduce(aep[:, 8 * b:8 * b + 8], tv,
                                            axis=AX.XY, op=ALU.add)
                if g < len(pre_tasks):
                    pre_tasks[g]()

            # ---------- tail: atom energies -> segments ----------
            ae8 = prepool.tile([128, 8], F32R, tag="ae8")
            nc.vector.tensor_reduce(ae8[:], aep[:, :].rearrange("p (b a) -> p a b", b=4),
                                    axis=AX.X, op=ALU.add)
            ps_seg = pspre.tile([1, NG], F32, tag="ps")
            for a in range(8):
                nc.tensor.matmul(ps_seg[:], ae8[:, a:a + 1], S[:, 64 * a:64 * (a + 1)],
                                 start=(a == 0), stop=(a == 7))
            nc.scalar.activation(out_sb[:, 0:64], ps_seg[:], AF.Copy)
            nc.sync.dma_start(d_out[:], out_sb[:])

    _legalize_waits(nc)
    nc._tile_ctx = tc_ref
    return nc


def kernel(**inputs) -> np.ndarray:
    f = lambda a: np.ascontiguousarray(np.asarray(a), dtype=np.float32)
    bf = lambda a: np.ascontiguousarray(np.asarray(a, dtype=np.float32)).astype(ml_dtypes.bfloat16)
    tf = f(inputs["token_features"])
    la = f(inputs["lig_atom"])
    lg = f(inputs["lig_graph"])
    m0 = f(inputs["ms_feat_0"])
    m1 = f(inputs["ms_feat_1"])
    lb = np.asarray(inputs["ligand_batch"])

    # one-hot segment matrix, atom-chunk-major: S[p, 64q+s] = [batch[128q+p]==s]
    S = (lb[:, None] == np.arange(NG)[None, :]).astype(np.float32)  # [1024, 64]
    Sh = np.zeros((128, 512), np.float32)
    for q in range(8):
        Sh[:, 64 * q:64 * (q + 1)] = S[128 * q:128 * (q + 1)]

    Wc1 = f(inputs["Wc1"])  # [128, 32, 3,3,3]
    Wc0 = f(inputs["Wc0"])  # [128, 64, 3,3,3]
    # W96[32*dx+c, 128*(3*dz+dy)+o] = Wc1[o,c,dz,dy,dx]
    W96 = Wc1.transpose(2, 3, 4, 1, 0).reshape(9, 96, HID)
    W96 = np.ascontiguousarray(W96.transpose(1, 0, 2).reshape(96, 9 * HID))
    W0t = Wc0.transpose(2, 3, 4, 1, 0).reshape(27, 64, HID)
    W0t = np.ascontiguousarray(W0t.transpose(1, 0, 2).reshape(64, 27 * HID))

    wcat = f(inputs["W_cat"]).copy()
    wgate = f(inputs["W_gate"]).copy()
    wcat[2 * HID:] /= 128.0   # token mean = sum / 128
    wgate[2 * HID:] /= 128.0

    # bf16 weight blob [128, BW_COLS]
    BW = np.zeros((128, BW_COLS), np.float32)
    BW[:, O_WINT:O_WINT + 128] = f(inputs["W_int"])
    BW[:, O_WTOK:O_WTOK + 256] = f(inputs["W_token"]).reshape(2, 128, HID).transpose(1, 0, 2).reshape(128, 256)
    BW[:, O_WPK:O_WPK + 256] = f(inputs["W_pocket"]).reshape(2, 128, HID).transpose(1, 0, 2).reshape(128, 256)
    BW[:, O_WCAT:O_WCAT + 384] = wcat.reshape(3, 128, HID).transpose(1, 0, 2).reshape(128, 384)
    BW[:, O_WGATE:O_WGATE + 384] = wgate.reshape(3, 128, HID).transpose(1, 0, 2).reshape(128, 384)
    BW[:, O_WB1:O_WB1 + 256] = f(inputs["W_bias1"]).reshape(2, 128, HID).transpose(1, 0, 2).reshape(128, 256)
    BW[0:64, O_WAG:O_WAG + 128] = f(inputs["W_atom"])
    BW[0:64, O_WG:O_WG + 128] = f(inputs["W_graph"])

    # f32 small blob [128, FW_COLS]
    col = lambda a: f(a).reshape(-1)
    FW = np.zeros((128, FW_COLS), np.float32)
    FW[:, C_WPE] = col(inputs["W_pe"])
    FW[:, C_WPG] = col(inputs["W_pg"])
    FW[:, C_WB2] = col(inputs["W_bias2"])
    FW[:, C_BTOK] = col(inputs["b_token"])
    FW[:, C_BPK] = col(inputs["b_pocket"])
    FW[:, C_BCAT] = col(inputs["b_cat"])
    FW[:, C_BGATEH] = col(inputs["b_gate"]) * 0.5
    FW[:, C_BATOM] = col(inputs["b_atom"])
    FW[:, C_BGRAPH] = col(inputs["b_graph"])
    FW[:, C_BB1] = col(inputs["b_bias1"])
    FW[:, C_BINT] = col(inputs["b_int"])
    wpe_v = f(inputs["W_pe"]).reshape(-1)
    wpg_v = f(inputs["W_pg"]).reshape(-1)
    wint_f = f(inputs["W_int"])
    bint_v = col(inputs["b_int"])
    FW[:, C_RPE] = 0.01 * (wint_f @ wpe_v)
    FW[:, C_RPG] = 0.01 * (wint_f @ wpg_v)
    bpe_eff = float(np.asarray(inputs["b_pe"]).reshape(-1)[0]) + 0.01 * float(wpe_v @ bint_v)
    bpg_eff = float(np.asarray(inputs["b_pg"]).reshape(-1)[0]) + 0.01 * float(wpg_v @ bint_v)
    FW[:, C_BPGH] = bpg_eff * 0.5
    FW[:, C_BB2] = float(np.asarray(inputs["b_bias2"]).reshape(-1)[0])

    Wpeg = 0.99 * np.concatenate([f(inputs["W_pe"]).reshape(128, 1),
                                  f(inputs["W_pg"]).reshape(128, 1)], axis=1)

    bpe = bpe_eff
    bpg = bpg_eff
    bb2 = float(np.asarray(inputs["b_bias2"]).reshape(-1)[0])

    shared = {
        "BW": BW.astype(ml_dtypes.bfloat16),
        "FW": FW,
        "Wpeg": Wpeg,
        "W96": W96.astype(ml_dtypes.bfloat16),
        "W0t": W0t.astype(ml_dtypes.bfloat16),
        "Sh": Sh,
    }

    in_maps = []
    for c in range(NCORES):
        n, h = c // 2, c % 2
        m = dict(shared)
        # permute tokens: this core's 64 first
        perm = np.concatenate([np.arange(64 * h, 64 * (h + 1)),
                               np.arange(64 * (1 - h), 64 * (2 - h))])
        tfp = tf[n][perm]                       # [128 tok, 256 feat]
        m["tfT"] = bf(np.ascontiguousarray(tfp.T))   # [256, 128] -> [128,256] view below
        # note: dram is [128, 256] = 2 chunks of features stacked on cols
        m["tfT"] = bf(np.ascontiguousarray(tfp.T.reshape(2, 128, 128).transpose(1, 0, 2).reshape(128, 256)))
        m["laT"] = bf(la[n].T)                  # [64, 1024]
        m["lgT"] = bf(lg[n].T)                  # [64, 64]
        m0f = m0[n].reshape(32, 4096)
        x3h = np.zeros((96, 4096), dtype=np.float32)
        for dd in range(3):
            x3h[32 * dd:32 * (dd + 1), 0:4096 - dd] = m0f[:, dd:]
        m["msf0"] = bf(x3h)
        m["msf1"] = bf(m1[n].reshape(64, 512))
        in_maps.append(m)

    bint_zero = bool(np.all(np.asarray(inputs['b_int']) == 0.0))
    nc = build_program(bpe, bpg, bb2, bint_zero)
    r = run_bass_kernel_spmd(nc, in_maps, core_ids=list(range(NCORES)),
                             trace=TRACE, **(TRACE_KW if TRACE else {}))
    global LAST
    LAST = r
    res = r.results

    out = np.zeros((NI, NG), dtype=np.float32)
    for n in range(NI):
        out[n] = (res[2 * n]["out"][0, 0:64] + res[2 * n + 1]["out"][0, 0:64]
                  + res[2 * n]["out"][0, 64:128])
    return out
